# revision 1
# baseline (speedup 1.0000x reference)
"""Correlation layer (FlowNet-style cost volume) Trainium2 Bass kernel.

out[b, o, h, w] = (1/C) * sum_c f1[b,c,h,w] * f2pad[b,c,h+dy,w+dx],
o = iy*21 + ix, (dy, dx) = (2*iy, 2*ix), zero padding 20 in H and W.
B=8, C=256, H=64, W=96, 441 offsets.  Data-parallel: one batch per core.

Per core:
  - W columns host-permuted even-first in both inputs; matmuls split by W
    parity (M=48) so cross-parity products (never needed: dx is even) are
    skipped and the band-extraction DMA access pattern stays legal.
  - PE: P[p, (t, c')] = sum_c' f1s[c', h, 2p+q] * f2p[c', h+2t, 2c'+q]
    (lhsT = f1 parity-half [128 x 48], rhs = f2p rows (4 dy batched),
    PSUM-accumulated over 2 C-chunks; 12 matmul groups per h row).
  - ScalarE: copy PSUM -> staging S[p, 68*t + 10 + c'].
  - Band extraction: ONE 3-dim diagonal-AP DMA per (h, parity):
    B[p, 21t + j] = S[p, 68t + p + j]  (flat step 1429 = pitch+1, legal),
    == correlation element (w = 2p+q, dy=2t, dx=2j); off-edge positions
    read unwritten staging and are masked to exact 0 on the host.
  - Output in [h, parity, p, o] device layout; host reassembles/transposes.
"""
import sys

for _p in ("/opt/trn_rl_repo", "/root/.axon_site/_ro/trn_rl_repo"):
    if _p not in sys.path:
        sys.path.insert(0, _p)

import numpy as np

import concourse.bass as bass
import concourse.mybir as mybir
from concourse.ap import AP
from concourse.bass_utils import run_bass_kernel_spmd

B, C, H, W = 8, 256, 64, 96
NOFF = 21
NCHUNK = 2
HP = H + 40
F1SZ = H * W                 # 6144
F2SZ = HP * W                # 9984
FIN = F1SZ + F2SZ            # 16128
SROW = NOFF * 68             # 1428 staging cols
NSLOT = 8                    # psum slots
GROUPS = [(0, 4), (4, 4), (8, 4), (12, 4), (16, 4), (20, 1)]  # (t0, ndy)

_nc_cache = None


def _build():
    nc = bass.Bass()
    fin = nc.declare_dram_parameter("fin", [128, NCHUNK * FIN], mybir.dt.float32,
                                    isOutput=False)
    out = nc.declare_dram_parameter("out", [H, 2, 48, NOFF * NOFF],
                                    mybir.dt.float32, isOutput=True)

    import contextlib
    ctx = contextlib.ExitStack()
    mega = ctx.enter_context(
        nc.sbuf_tensor("mega", [128, NCHUNK * FIN], mybir.dt.float32))
    S = [[ctx.enter_context(nc.sbuf_tensor(f"S{q}{i}", [48, SROW],
                                           mybir.dt.float32))
          for i in range(2)] for q in range(2)]
    Bt = [[ctx.enter_context(nc.sbuf_tensor(f"Bt{q}{i}", [48, NOFF * NOFF],
                                            mybir.dt.float32))
           for i in range(2)] for q in range(2)]
    slots = [ctx.enter_context(nc.psum_tensor(f"slot{s}", [48, 192],
                                              mybir.dt.float32))
             for s in range(NSLOT)]

    load_sem = ctx.enter_context(nc.semaphore("load_sem"))
    pe_sem = ctx.enter_context(nc.semaphore("pe_sem"))
    cp_sem = ctx.enter_context(nc.semaphore("cp_sem"))
    band_sem = [ctx.enter_context(nc.semaphore(f"band{q}")) for q in range(2)]
    outq_sem = [ctx.enter_context(nc.semaphore(f"outq{q}")) for q in range(2)]

    def lhsT_ap(ch, h, q):
        return AP(tensor=mega, offset=ch * FIN + h * W + q * 48,
                  ap=[[NCHUNK * FIN, 128], [1, 48]])

    def rhs_ap(ch, h, q, t0, gn):
        off = ch * FIN + F1SZ + (h + 2 * t0) * W + q * 48
        return AP(tensor=mega, offset=off,
                  ap=[[NCHUNK * FIN, 128], [2 * W, gn], [1, 48]])

    def slot_out_ap(s, gn):
        return AP(tensor=slots[s], offset=0, ap=[[192, 48], [1, gn * 48]])

    def slot_rd_ap(s, gn):
        return AP(tensor=slots[s], offset=0, ap=[[192, 48], [48, gn], [1, 48]])

    def stage_wr_ap(q, hb, t0, gn):
        return AP(tensor=S[q][hb], offset=68 * t0 + 10,
                  ap=[[SROW, 48], [68, gn], [1, 48]])

    # matmul groups in program order
    sched = [(h, q, gi) for h in range(H) for q in range(2)
             for gi in range(len(GROUPS))]

    with nc.Block() as block:
        @block.tensor
        def _(tensor):
            tensor.wait_ge(load_sem, 16)
            for idx, (h, q, gi) in enumerate(sched):
                t0, gn = GROUPS[gi]
                s = idx % NSLOT
                if idx >= NSLOT:
                    tensor.wait_ge(cp_sem, idx - NSLOT + 1)
                for ch in range(NCHUNK):
                    mm = tensor.matmul(
                        slot_out_ap(s, gn),
                        lhsT_ap(ch, h, q),
                        rhs_ap(ch, h, q, t0, gn),
                        start=(ch == 0),
                        stop=(ch == NCHUNK - 1),
                    )
                    if ch == NCHUNK - 1:
                        mm.then_inc(pe_sem, 1)

        @block.scalar
        def _(scalar):
            for idx, (h, q, gi) in enumerate(sched):
                t0, gn = GROUPS[gi]
                s = idx % NSLOT
                if gi == 0 and h >= 2:
                    scalar.wait_ge(band_sem[q], 16 * (h - 1))
                scalar.wait_ge(pe_sem, idx + 1)
                scalar.copy(stage_wr_ap(q, h % 2, t0, gn),
                            slot_rd_ap(s, gn)).then_inc(cp_sem, 1)

        def q_engine_body(eng, q):
            with nc.allow_non_contiguous_dma(reason="band diag extraction"):
                for h in range(H):
                    eng.wait_ge(cp_sem, 12 * h + 6 * (q + 1))
                    if h >= 2:
                        eng.wait_ge(outq_sem[q], 16 * (h - 1))
                    src = AP(tensor=S[q][h % 2], offset=0,
                             ap=[[SROW + 1, 48], [68, NOFF], [1, NOFF]])
                    dst = AP(tensor=Bt[q][h % 2], offset=0,
                             ap=[[441, 48], [NOFF, NOFF], [1, NOFF]])
                    eng.dma_start(out=dst, in_=src).then_inc(band_sem[q], 16)
                    eng.wait_ge(band_sem[q], 16 * (h + 1))
                    eng.dma_start(out=out[h, q],
                                  in_=Bt[q][h % 2][:, :]).then_inc(outq_sem[q], 16)
                eng.wait_ge(outq_sem[q], 16 * H)

        @block.sync
        def _(sync):
            sync.dma_start(out=mega[:, :], in_=fin[:, :]).then_inc(load_sem, 16)
            q_engine_body(sync, 0)

        @block.gpsimd
        def _(gpsimd):
            q_engine_body(gpsimd, 1)

    return nc


def _get_nc():
    global _nc_cache
    if _nc_cache is None:
        _nc_cache = _build()
    return _nc_cache


def kernel(features_1: np.ndarray, features_2: np.ndarray) -> np.ndarray:
    f1 = np.asarray(features_1, dtype=np.float32)
    f2 = np.asarray(features_2, dtype=np.float32)
    assert f1.shape == (B, C, H, W) and f2.shape == (B, C, H, W)

    nc = _get_nc()
    wperm = np.concatenate([np.arange(0, W, 2), np.arange(1, W, 2)])
    in_maps = []
    for b in range(B):
        f1p = (f1[b] * np.float32(1.0 / C))[:, :, wperm]
        f1b = f1p.reshape(NCHUNK, 128, F1SZ)
        f2b = np.pad(f2[b], ((0, 0), (20, 20), (0, 0)))[:, :, wperm]
        f2b = f2b.reshape(NCHUNK, 128, F2SZ)
        finb = np.concatenate([f1b[0], f2b[0], f1b[1], f2b[1]], axis=1)
        in_maps.append({"fin": np.ascontiguousarray(finb)})

    res = run_bass_kernel_spmd(nc, in_maps, list(range(B)))

    # valid iff 0 <= w + 2j - 20 < W  (else window off the zero padding -> 0)
    wv = np.arange(W)[:, None]
    jv = np.tile(np.arange(NOFF), NOFF)[None, :]
    mask = ((wv + 2 * jv - 20 >= 0) & (wv + 2 * jv - 20 < W)).astype(np.float32)
    outs = []
    for b in range(B):
        o = res.results[b]["out"]            # [64, 2, 48, 441]
        full = np.empty((H, W, NOFF * NOFF), np.float32)
        full[:, 0::2, :] = o[:, 0]
        full[:, 1::2, :] = o[:, 1]
        full *= mask[None, :, :]
        outs.append(np.transpose(full, (2, 0, 1)))
    return np.stack(outs).astype(np.float32)



# revision 10
# speedup vs baseline: 2482.6297x; 2482.6297x over previous
"""Correlation layer (FlowNet-style cost volume) Trainium2 Bass kernel.

out[b, o, h, w] = (1/C) * sum_c f1[b,c,h,w] * f2pad[b,c,h+dy,w+dx],
o = iy*21 + ix, (dy, dx) = (2*iy, 2*ix), zero padding 20 in H and W.
B=8, C=256, H=64, W=96, 441 offsets.  Data-parallel: one batch per core.

Per core (all bf16 on the wire and in SBUF; PSUM accumulation fp32):
  - Inputs packed host-side as fin [128, 4*6144] = [f1c0|f1c1|f2c0|f2c1],
    W columns permuted even-first so matmuls split by W parity (M=48)
    skip cross-parity products (dx is always even).
  - Device pads f2 in H (zero margin rows memset once; middle rows DMAed
    straight from DRAM), so the padded rows are never uploaded.
  - PE: psum[p, (t, c')] = sum_c f1s[c, h, 2p+q] * f2p[c, h+2t, 2c'+q],
    10 dy rows batched per matmul (PSUM bank holds 480 fp32), 3 groups
    per (h, parity), accumulated over 2 C-chunks with ch-grouped issue
    order so the stationary f1 tile is loaded twice per (h, parity).
  - PSUM -> staging copies (fp32->bf16 cast) write c'-major:
    S[p, (c'+10)*21 + t], split across ScalarE (q=0) and VectorE (q=1).
  - Band extraction: the 441 needed correlations of partition p are the
    CONTIGUOUS staging run S[p, 21p : 21p+441] (element r = j*21 + t ==
    (dy=2t, dx=2j) at w=2p+q); staging has 4 h-slots, so ONE diagonal-AP
    DMA per (4 h, parity) ships straight to DRAM out (48*4 descriptors
    of 882B). Off-edge positions read staging margins memset to 0 once
    -> output exact 0, no host-side masking.
  - Output DRAM [h, parity, p, j*21+t] bf16; host reassembles/upcasts.

Execution avoids run_bass_via_pjrt's per-call zero-output upload: the
output-buffer operands live on device once and are NOT donated (the
kernel writes every output element, so their values never matter).
"""
import sys

for _p in ("/opt/trn_rl_repo", "/root/.axon_site/_ro/trn_rl_repo"):
    if _p not in sys.path:
        sys.path.insert(0, _p)

import numpy as np
import ml_dtypes

import concourse.bass as bass
import concourse.mybir as mybir
from concourse.ap import AP

BF16 = ml_dtypes.bfloat16

B, C, H, W = 8, 256, 64, 96
NOFF = 21
NCHUNK = 2
HP = H + 40                  # 104 padded f2 rows
F1SZ = H * W                 # 6144 per chunk
F2SZ = HP * W                # 9984 per chunk (padded, device side)
FINROW = 4 * F1SZ            # 24576: [f1c0|f1c1|f2c0|f2c1]
SROW = NOFF * 68             # 1428 staging cols per h-slot
HSLOT = 4                    # h-slots per staging tensor
NSLOT = 8                    # psum slots
GROUPS = [(0, 10), (10, 10), (20, 1)]  # (t0, ndy)
NG = len(GROUPS)

_rt_cache = None


def _build():
    nc = bass.Bass()
    fin = nc.declare_dram_parameter("fin", [128, FINROW], mybir.dt.bfloat16,
                                    isOutput=False)
    out = nc.declare_dram_parameter("out", [H, 2, 48, NOFF * NOFF],
                                    mybir.dt.bfloat16, isOutput=True)

    import contextlib
    ctx = contextlib.ExitStack()
    f1s = ctx.enter_context(
        nc.sbuf_tensor("f1s", [128, NCHUNK * F1SZ], mybir.dt.bfloat16))
    f2p = ctx.enter_context(
        nc.sbuf_tensor("f2p", [128, NCHUNK * F2SZ], mybir.dt.bfloat16))
    S = [ctx.enter_context(nc.sbuf_tensor(f"S{q}", [48, HSLOT * SROW],
                                          mybir.dt.bfloat16))
         for q in range(2)]
    slots = [ctx.enter_context(nc.psum_tensor(f"slot{s}", [48, 480],
                                              mybir.dt.float32))
             for s in range(NSLOT)]

    load_sem = ctx.enter_context(nc.semaphore("load_sem"))
    init_sem = ctx.enter_context(nc.semaphore("init_sem"))
    pe_sem = ctx.enter_context(nc.semaphore("pe_sem"))
    cp_sem = [ctx.enter_context(nc.semaphore(f"cp{q}")) for q in range(2)]
    band_sem = [ctx.enter_context(nc.semaphore(f"band{q}")) for q in range(2)]

    def lhsT_ap(ch, h, q):
        return AP(tensor=f1s, offset=ch * F1SZ + h * W + q * 48,
                  ap=[[NCHUNK * F1SZ, 128], [1, 48]])

    def rhs_ap(ch, h, q, t0, gn):
        off = ch * F2SZ + (h + 2 * t0) * W + q * 48
        return AP(tensor=f2p, offset=off,
                  ap=[[NCHUNK * F2SZ, 128], [2 * W, gn], [1, 48]])

    def slot_out_ap(s, gn):
        return AP(tensor=slots[s], offset=0, ap=[[480, 48], [1, gn * 48]])

    def slot_rd_ap(s, gn):
        # iterate (partition, c', t): innermost contiguous on the dst side
        return AP(tensor=slots[s], offset=0,
                  ap=[[480, 48], [1, 48], [48, gn]])

    def stage_wr_ap(q, hb, t0, gn):
        # S[p, (c'+10)*21 + t] for t in [t0, t0+gn), c' in [0,48)
        return AP(tensor=S[q], offset=hb * SROW + 10 * NOFF + t0,
                  ap=[[HSLOT * SROW, 48], [NOFF, 48], [1, gn]])

    # global matmul-group schedule: (h, q, gi)
    sched = [(h, q, gi) for h in range(H) for q in range(2)
             for gi in range(NG)]

    def copy_sem_target(g):
        """(sem_q, count): completion of the staging copy for group g."""
        h, q, gi = g // (2 * NG), (g // NG) % 2, g % NG
        return q, h * NG + gi + 1

    with nc.Block() as block:
        @block.tensor
        def _(tensor):
            tensor.wait_ge(load_sem, 48)   # f1 + 2 f2 chunk DMAs
            tensor.wait_ge(init_sem, 1)    # f2 H-margin + staging memsets
            for hq in range(H * 2):
                h, q = hq // 2, hq % 2
                g0 = hq * NG
                for ch in range(NCHUNK):
                    for gi, (t0, gn) in enumerate(GROUPS):
                        g = g0 + gi
                        s = g % NSLOT
                        if ch == 0 and g >= NSLOT:
                            qq, cnt = copy_sem_target(g - NSLOT)
                            tensor.wait_ge(cp_sem[qq], cnt)
                        mm = tensor.matmul(
                            slot_out_ap(s, gn),
                            lhsT_ap(ch, h, q),
                            rhs_ap(ch, h, q, t0, gn),
                            start=(ch == 0),
                            stop=(ch == NCHUNK - 1),
                        )
                        if ch == NCHUNK - 1:
                            mm.then_inc(pe_sem, 1)

        def copy_engine_body(eng, q):
            do_copy = getattr(eng, "copy", None) or eng.tensor_copy
            for h in range(H):
                for gi, (t0, gn) in enumerate(GROUPS):
                    g = (h * 2 + q) * NG + gi
                    s = g % NSLOT
                    if gi == 0 and h % HSLOT == 0 and h >= HSLOT:
                        eng.wait_ge(band_sem[q], 16 * (h // HSLOT))
                    eng.wait_ge(pe_sem, g + 1)
                    do_copy(stage_wr_ap(q, h % HSLOT, t0, gn),
                            slot_rd_ap(s, gn)).then_inc(cp_sem[q], 1)

        @block.scalar
        def _(scalar):
            copy_engine_body(scalar, 0)

        @block.vector
        def _(vector):
            # zero f2 H-padding margins (20 rows top/bottom per chunk) and
            # the staging (margins + everything) once: band extraction then
            # reads exact 0 at off-edge positions.
            for ch in range(NCHUNK):
                vector.memset(AP(tensor=f2p, offset=ch * F2SZ,
                                 ap=[[NCHUNK * F2SZ, 128], [1, 20 * W]]), 0.0)
                vector.memset(AP(tensor=f2p, offset=ch * F2SZ + (20 + H) * W,
                                 ap=[[NCHUNK * F2SZ, 128], [1, 20 * W]]), 0.0)
            memsets = [vector.memset(S[q][:, :], 0.0) for q in range(2)]
            memsets[-1].then_inc(init_sem, 1)
            copy_engine_body(vector, 1)

        def q_band_body(eng, q):
            with nc.allow_non_contiguous_dma(reason="band diag extraction"):
                for k in range(H // HSLOT):
                    if k == 0:
                        eng.wait_ge(init_sem, 1)  # staging margins zeroed
                    eng.wait_ge(cp_sem[q], NG * HSLOT * (k + 1))
                    src = AP(tensor=S[q], offset=0,
                             ap=[[HSLOT * SROW + NOFF, 48], [SROW, HSLOT],
                                 [1, NOFF * NOFF]])
                    dst = AP(tensor=out, offset=(HSLOT * k * 2 + q) * 48 * 441,
                             ap=[[441, 48], [2 * 48 * 441, HSLOT], [1, 441]])
                    eng.dma_start(out=dst, in_=src).then_inc(band_sem[q], 16)
                eng.wait_ge(band_sem[q], 16 * (H // HSLOT))

        @block.sync
        def _(sync):
            sync.dma_start(out=f1s[:, :],
                           in_=fin[:, 0:NCHUNK * F1SZ]).then_inc(load_sem, 16)
            for ch in range(NCHUNK):
                dst = AP(tensor=f2p, offset=ch * F2SZ + 20 * W,
                         ap=[[NCHUNK * F2SZ, 128], [1, F1SZ]])
                src = fin[:, (2 + ch) * F1SZ:(3 + ch) * F1SZ]
                sync.dma_start(out=dst, in_=src).then_inc(load_sem, 16)
            q_band_body(sync, 0)

        @block.gpsimd
        def _(gpsimd):
            q_band_body(gpsimd, 1)

    return nc


def _pack_inputs(f1: np.ndarray, f2: np.ndarray) -> np.ndarray:
    """Full fp32 inputs [8,256,64,96] -> fin global [8*128, FINROW] bf16."""
    inv_c = np.float32(1.0 / C)
    fin = np.empty((B, 128, FINROW), dtype=BF16)
    finv = fin.reshape(B, 128, 4, F1SZ)
    f1r = f1.reshape(B, 2, 128, H, W)
    f2r = f2.reshape(B, 2, 128, H, W)
    fv = finv.reshape(B, 128, 4, H, 2, W // 2)
    for ch in range(2):
        fs = f1r[:, ch] * inv_c                       # [B,128,H,W] fp32
        fv[:, :, ch, :, 0, :] = fs[:, :, :, 0::2]     # cast on assign
        fv[:, :, ch, :, 1, :] = fs[:, :, :, 1::2]
        fv[:, :, 2 + ch, :, 0, :] = f2r[:, ch][:, :, :, 0::2]
        fv[:, :, 2 + ch, :, 1, :] = f2r[:, ch][:, :, :, 1::2]
    return fin.reshape(B * 128, FINROW)


def _unpack_outputs(shards: list[np.ndarray]) -> np.ndarray:
    """Per-core [64, 2, 48, 441] bf16 (441 = j*21+t) -> [8,441,64,96] fp32."""
    outs = np.empty((B, NOFF * NOFF, H, W), np.float32)
    for b, o in enumerate(shards):
        ov = o.reshape(H, 2, 48, NOFF, NOFF)          # [h, q, p, j, t]
        outs[b] = ov.transpose(4, 3, 0, 2, 1).astype(np.float32).reshape(
            NOFF * NOFF, H, W)                        # o = t*21+j, w = 2p+q
    return outs


class _Runtime:
    def __init__(self):
        import jax
        from jax.experimental.shard_map import shard_map
        from jax.sharding import Mesh, PartitionSpec, NamedSharding
        from concourse.bass2jax import install_neuronx_cc_hook, _bass_exec_p
        from concourse.bass2jax import partition_id_tensor

        self.jax = jax
        install_neuronx_cc_hook()
        nc = self.nc = _build()

        in_names, out_names, out_avals, out_shapes = [], [], [], []
        for alloc in nc.m.functions[0].allocations:
            if not isinstance(alloc, mybir.MemoryLocationSet):
                continue
            name = alloc.memorylocations[0].name
            if alloc.kind == "ExternalInput":
                if (nc.partition_id_tensor is None
                        or name != nc.partition_id_tensor.name):
                    in_names.append(name)
            elif alloc.kind == "ExternalOutput":
                shape = tuple(alloc.tensor_shape)
                out_names.append(name)
                out_shapes.append(shape)
                out_avals.append(
                    jax.core.ShapedArray(shape, mybir.dt.np(alloc.dtype)))
        n_params = len(in_names)
        in_names = in_names + out_names
        if nc.partition_id_tensor is not None:
            in_names.append(nc.partition_id_tensor.name)

        def _body(*args):
            operands = list(args)
            if nc.partition_id_tensor is not None:
                operands.append(partition_id_tensor())
            outs = _bass_exec_p.bind(
                *operands,
                out_avals=tuple(out_avals),
                in_names=tuple(in_names),
                out_names=tuple(out_names),
                lowering_input_output_aliases=(),
                sim_require_finite=True,
                sim_require_nnan=True,
                nc=nc,
            )
            return tuple(outs)

        devices = jax.devices()[:B]
        assert len(devices) == B
        mesh = Mesh(np.asarray(devices), ("core",))
        self.sharding = NamedSharding(mesh, PartitionSpec("core"))
        nin = n_params + len(out_names)
        self.sharded = jax.jit(
            shard_map(_body, mesh=mesh,
                      in_specs=(PartitionSpec("core"),) * nin,
                      out_specs=(PartitionSpec("core"),) * len(out_names),
                      check_rep=False),
            keep_unused=True,
        )
        # Persistent (non-donated) output-buffer operands: the kernel writes
        # every element of out, so these are never observed.
        self.outbuf = jax.device_put(
            np.zeros((B * out_shapes[0][0],) + out_shapes[0][1:], BF16),
            self.sharding)

    def run(self, fin_global: np.ndarray) -> list[np.ndarray]:
        jax = self.jax
        fin_dev = jax.device_put(fin_global, self.sharding)
        (out_g,) = self.sharded(fin_dev, self.outbuf)
        shards = sorted(out_g.addressable_shards,
                        key=lambda s: s.index[0].start or 0)
        for sh in shards:
            sh.data.copy_to_host_async()
        return [np.asarray(sh.data) for sh in shards]


def _get_rt() -> "_Runtime":
    global _rt_cache
    if _rt_cache is None:
        _rt_cache = _Runtime()
    return _rt_cache


def kernel(features_1: np.ndarray, features_2: np.ndarray) -> np.ndarray:
    f1 = np.asarray(features_1, dtype=np.float32)
    f2 = np.asarray(features_2, dtype=np.float32)
    assert f1.shape == (B, C, H, W) and f2.shape == (B, C, H, W)
    rt = _get_rt()
    fin = _pack_inputs(f1, f2)
    shards = rt.run(fin)
    return _unpack_outputs(shards)


# revision 12
# speedup vs baseline: 9199.9363x; 3.7057x over previous
"""Correlation layer (FlowNet-style cost volume) Trainium2 Bass kernel.

out[b, o, h, w] = (1/C) * sum_c f1[b,c,h,w] * f2pad[b,c,h+dy,w+dx],
o = iy*21 + ix, (dy, dx) = (2*iy, 2*ix), zero padding 20 in H and W.
B=8, C=256, H=64, W=96, 441 offsets.  Data-parallel: one batch per core.

Per core (all bf16 on the wire and in SBUF; PSUM accumulation fp32):
  - Inputs packed host-side as fin [128, 4*6144] = [f1c0|f1c1|f2c0|f2c1],
    W columns permuted even-first so matmuls split by W parity (M=48)
    skip cross-parity products (dx is always even).
  - Device pads f2 in H (zero margin rows memset once; middle rows DMAed
    straight from DRAM), so the padded rows are never uploaded.
  - PE: psum[p, (t, c')] = sum_c f1s[c, h, 2p+q] * f2p[c, h+2t, 2c'+q],
    10 dy rows batched per matmul (PSUM bank holds 480 fp32), 3 groups
    per (h, parity), accumulated over 2 C-chunks with ch-grouped issue
    order so the stationary f1 tile is loaded twice per (h, parity).
  - PSUM -> staging copies (fp32->bf16 cast) write c'-major:
    S[p, (c'+10)*21 + t], split across ScalarE (q=0) and VectorE (q=1).
  - Band extraction: the 441 needed correlations of partition p are the
    CONTIGUOUS staging run S[p, 21p : 21p+441] (element r = j*21 + t ==
    (dy=2t, dx=2j) at w=2p+q); staging has 4 h-slots, so ONE diagonal-AP
    DMA per (4 h, parity) ships straight to DRAM out (48*4 descriptors
    of 882B). Off-edge positions read staging margins memset to 0 once
    -> output exact 0, no host-side masking.
  - Output DRAM [h, parity, p, j*21+t] bf16; host reassembles/upcasts.

Execution avoids run_bass_via_pjrt's per-call zero-output upload: the
output-buffer operands live on device once and are NOT donated (the
kernel writes every output element, so their values never matter).
"""
import sys

for _p in ("/opt/trn_rl_repo", "/root/.axon_site/_ro/trn_rl_repo"):
    if _p not in sys.path:
        sys.path.insert(0, _p)

import numpy as np
import ml_dtypes

import concourse.bass as bass
import concourse.mybir as mybir
from concourse.ap import AP

BF16 = ml_dtypes.bfloat16

B, C, H, W = 8, 256, 64, 96
NOFF = 21
NCHUNK = 2
HP = H + 40                  # 104 padded f2 rows
F1SZ = H * W                 # 6144 per chunk
F2SZ = HP * W                # 9984 per chunk (padded, device side)
FINROW = 4 * F1SZ            # 24576: [f1c0|f1c1|f2c0|f2c1]
SROW = NOFF * 68             # 1428 staging cols per h-slot
HSLOT = 4                    # h-slots per staging tensor
NSLOT = 8                    # psum slots
GROUPS = [(0, 10), (10, 10), (20, 1)]  # (t0, ndy)
NG = len(GROUPS)

_rt_cache = None


def _build():
    nc = bass.Bass()
    fin = nc.declare_dram_parameter("fin", [128, FINROW], mybir.dt.bfloat16,
                                    isOutput=False)
    out = nc.declare_dram_parameter("out", [H, 2, 48, NOFF * NOFF],
                                    mybir.dt.bfloat16, isOutput=True)

    import contextlib
    ctx = contextlib.ExitStack()
    f1s = ctx.enter_context(
        nc.sbuf_tensor("f1s", [128, NCHUNK * F1SZ], mybir.dt.bfloat16))
    f2p = ctx.enter_context(
        nc.sbuf_tensor("f2p", [128, NCHUNK * F2SZ], mybir.dt.bfloat16))
    S = [ctx.enter_context(nc.sbuf_tensor(f"S{q}", [48, HSLOT * SROW],
                                          mybir.dt.bfloat16))
         for q in range(2)]
    slots = [ctx.enter_context(nc.psum_tensor(f"slot{s}", [48, 480],
                                              mybir.dt.float32))
             for s in range(NSLOT)]

    load_sem = ctx.enter_context(nc.semaphore("load_sem"))
    init_sem = ctx.enter_context(nc.semaphore("init_sem"))
    pe_sem = ctx.enter_context(nc.semaphore("pe_sem"))
    cp_sem = [ctx.enter_context(nc.semaphore(f"cp{q}")) for q in range(2)]
    band_sem = [ctx.enter_context(nc.semaphore(f"band{q}")) for q in range(2)]

    def lhsT_ap(ch, h, q):
        return AP(tensor=f1s, offset=ch * F1SZ + h * W + q * 48,
                  ap=[[NCHUNK * F1SZ, 128], [1, 48]])

    def rhs_ap(ch, h, q, t0, gn):
        off = ch * F2SZ + (h + 2 * t0) * W + q * 48
        return AP(tensor=f2p, offset=off,
                  ap=[[NCHUNK * F2SZ, 128], [2 * W, gn], [1, 48]])

    def slot_out_ap(s, gn):
        return AP(tensor=slots[s], offset=0, ap=[[480, 48], [1, gn * 48]])

    def slot_rd_ap(s, gn):
        # iterate (partition, c', t): innermost contiguous on the dst side
        return AP(tensor=slots[s], offset=0,
                  ap=[[480, 48], [1, 48], [48, gn]])

    def stage_wr_ap(q, hb, t0, gn):
        # S[p, (c'+10)*21 + t] for t in [t0, t0+gn), c' in [0,48)
        return AP(tensor=S[q], offset=hb * SROW + 10 * NOFF + t0,
                  ap=[[HSLOT * SROW, 48], [NOFF, 48], [1, gn]])

    # global matmul-group schedule: (h, q, gi)
    sched = [(h, q, gi) for h in range(H) for q in range(2)
             for gi in range(NG)]

    def copy_sem_target(g):
        """(sem_q, count): completion of the staging copy for group g."""
        h, q, gi = g // (2 * NG), (g // NG) % 2, g % NG
        return q, h * NG + gi + 1

    with nc.Block() as block:
        @block.tensor
        def _(tensor):
            tensor.wait_ge(load_sem, 48)   # f1 + 2 f2 chunk DMAs
            tensor.wait_ge(init_sem, 1)    # f2 H-margin + staging memsets
            for hq in range(H * 2):
                h, q = hq // 2, hq % 2
                g0 = hq * NG
                for ch in range(NCHUNK):
                    for gi, (t0, gn) in enumerate(GROUPS):
                        g = g0 + gi
                        s = g % NSLOT
                        if ch == 0 and g >= NSLOT:
                            qq, cnt = copy_sem_target(g - NSLOT)
                            tensor.wait_ge(cp_sem[qq], cnt)
                        mm = tensor.matmul(
                            slot_out_ap(s, gn),
                            lhsT_ap(ch, h, q),
                            rhs_ap(ch, h, q, t0, gn),
                            start=(ch == 0),
                            stop=(ch == NCHUNK - 1),
                        )
                        if ch == NCHUNK - 1:
                            mm.then_inc(pe_sem, 1)

        def copy_engine_body(eng, q):
            do_copy = getattr(eng, "copy", None) or eng.tensor_copy
            for h in range(H):
                for gi, (t0, gn) in enumerate(GROUPS):
                    g = (h * 2 + q) * NG + gi
                    s = g % NSLOT
                    if gi == 0 and h % HSLOT == 0 and h >= HSLOT:
                        eng.wait_ge(band_sem[q], 16 * (h // HSLOT))
                    eng.wait_ge(pe_sem, g + 1)
                    do_copy(stage_wr_ap(q, h % HSLOT, t0, gn),
                            slot_rd_ap(s, gn)).then_inc(cp_sem[q], 1)

        @block.scalar
        def _(scalar):
            copy_engine_body(scalar, 0)

        @block.vector
        def _(vector):
            # zero f2 H-padding margins (20 rows top/bottom per chunk) and
            # the staging (margins + everything) once: band extraction then
            # reads exact 0 at off-edge positions.
            for ch in range(NCHUNK):
                vector.memset(AP(tensor=f2p, offset=ch * F2SZ,
                                 ap=[[NCHUNK * F2SZ, 128], [1, 20 * W]]), 0.0)
                vector.memset(AP(tensor=f2p, offset=ch * F2SZ + (20 + H) * W,
                                 ap=[[NCHUNK * F2SZ, 128], [1, 20 * W]]), 0.0)
            memsets = [vector.memset(S[q][:, :], 0.0) for q in range(2)]
            memsets[-1].then_inc(init_sem, 1)
            copy_engine_body(vector, 1)

        def q_band_body(eng, q):
            with nc.allow_non_contiguous_dma(reason="band diag extraction"):
                for k in range(H // HSLOT):
                    if k == 0:
                        eng.wait_ge(init_sem, 1)  # staging margins zeroed
                    eng.wait_ge(cp_sem[q], NG * HSLOT * (k + 1))
                    src = AP(tensor=S[q], offset=0,
                             ap=[[HSLOT * SROW + NOFF, 48], [SROW, HSLOT],
                                 [1, NOFF * NOFF]])
                    dst = AP(tensor=out, offset=(HSLOT * k * 2 + q) * 48 * 441,
                             ap=[[441, 48], [2 * 48 * 441, HSLOT], [1, 441]])
                    eng.dma_start(out=dst, in_=src).then_inc(band_sem[q], 16)
                eng.wait_ge(band_sem[q], 16 * (H // HSLOT))

        @block.sync
        def _(sync):
            sync.dma_start(out=f1s[:, :],
                           in_=fin[:, 0:NCHUNK * F1SZ]).then_inc(load_sem, 16)
            for ch in range(NCHUNK):
                dst = AP(tensor=f2p, offset=ch * F2SZ + 20 * W,
                         ap=[[NCHUNK * F2SZ, 128], [1, F1SZ]])
                src = fin[:, (2 + ch) * F1SZ:(3 + ch) * F1SZ]
                sync.dma_start(out=dst, in_=src).then_inc(load_sem, 16)
            q_band_body(sync, 0)

        @block.gpsimd
        def _(gpsimd):
            q_band_body(gpsimd, 1)

    return nc


_host_fns = None


def _get_host_fns():
    """Jitted XLA-CPU pack/unpack (numpy's strided bf16 casts are ~10x
    slower than XLA's fused multithreaded gather+cast)."""
    global _host_fns
    if _host_fns is None:
        import jax
        import jax.numpy as jnp
        cpu = jax.devices("cpu")[0]

        def pack_fn(f1, f2):
            fs = f1.reshape(B, 2, 128, H, W) * np.float32(1.0 / C)
            f2r = f2.reshape(B, 2, 128, H, W)

            def parts(x):  # [B,ch,128,H,W] -> [B,ch,128,H,2par,48]
                return jnp.stack([x[..., 0::2], x[..., 1::2]], axis=4)

            finv = jnp.concatenate([parts(fs), parts(f2r)], axis=1)
            finv = finv.transpose(0, 2, 1, 3, 4, 5)   # [B,128,4,H,2,48]
            return finv.reshape(B * 128, FINROW).astype(jnp.bfloat16)

        def unpack_fn(o):  # [B,64,2,48,441] bf16 -> [B,441,64,96] fp32
            ov = o.reshape(B, H, 2, 48, NOFF, NOFF).astype(jnp.float32)
            return ov.transpose(0, 5, 4, 1, 3, 2).reshape(
                B, NOFF * NOFF, H, W)  # o = t*21+j, w = 2p+q

        _host_fns = (jax.jit(pack_fn, device=cpu),
                     jax.jit(unpack_fn, device=cpu))
    return _host_fns


def _pack_inputs(f1: np.ndarray, f2: np.ndarray):
    """Full fp32 inputs [8,256,64,96] -> fin global [8*128, FINROW] bf16."""
    pack_fn, _ = _get_host_fns()
    return pack_fn(f1, f2)


def _unpack_outputs(shards: list[np.ndarray]) -> np.ndarray:
    """Per-core [64, 2, 48, 441] bf16 (441 = j*21+t) -> [8,441,64,96] fp32."""
    _, unpack_fn = _get_host_fns()
    return np.asarray(unpack_fn(np.stack(shards)))


class _Runtime:
    def __init__(self):
        import jax
        from jax.experimental.shard_map import shard_map
        from jax.sharding import Mesh, PartitionSpec, NamedSharding
        from concourse.bass2jax import install_neuronx_cc_hook, _bass_exec_p
        from concourse.bass2jax import partition_id_tensor

        self.jax = jax
        install_neuronx_cc_hook()
        nc = self.nc = _build()

        in_names, out_names, out_avals, out_shapes = [], [], [], []
        for alloc in nc.m.functions[0].allocations:
            if not isinstance(alloc, mybir.MemoryLocationSet):
                continue
            name = alloc.memorylocations[0].name
            if alloc.kind == "ExternalInput":
                if (nc.partition_id_tensor is None
                        or name != nc.partition_id_tensor.name):
                    in_names.append(name)
            elif alloc.kind == "ExternalOutput":
                shape = tuple(alloc.tensor_shape)
                out_names.append(name)
                out_shapes.append(shape)
                out_avals.append(
                    jax.core.ShapedArray(shape, mybir.dt.np(alloc.dtype)))
        n_params = len(in_names)
        in_names = in_names + out_names
        if nc.partition_id_tensor is not None:
            in_names.append(nc.partition_id_tensor.name)

        def _body(*args):
            operands = list(args)
            if nc.partition_id_tensor is not None:
                operands.append(partition_id_tensor())
            outs = _bass_exec_p.bind(
                *operands,
                out_avals=tuple(out_avals),
                in_names=tuple(in_names),
                out_names=tuple(out_names),
                lowering_input_output_aliases=(),
                sim_require_finite=True,
                sim_require_nnan=True,
                nc=nc,
            )
            return tuple(outs)

        devices = jax.devices()[:B]
        assert len(devices) == B
        mesh = Mesh(np.asarray(devices), ("core",))
        self.sharding = NamedSharding(mesh, PartitionSpec("core"))
        nin = n_params + len(out_names)

        def _make_jit():
            return jax.jit(
                shard_map(_body, mesh=mesh,
                          in_specs=(PartitionSpec("core"),) * nin,
                          out_specs=(PartitionSpec("core"),) * len(out_names),
                          check_rep=False),
                keep_unused=True,
            )

        # Prefer the C++ fast-dispatch path (no effects token): lower with
        # concrete shardings and compile under _fast_dispatch_active.
        try:
            from concourse.bass2jax import fast_dispatch_compile
            fin_sds = jax.ShapeDtypeStruct((B * 128, FINROW), BF16,
                                           sharding=self.sharding)
            out_sds = jax.ShapeDtypeStruct(
                (B * out_shapes[0][0],) + out_shapes[0][1:], BF16,
                sharding=self.sharding)
            self.sharded = fast_dispatch_compile(
                lambda: _make_jit().lower(fin_sds, out_sds).compile())
        except Exception:
            self.sharded = _make_jit()
        # Persistent (non-donated) output-buffer operands: the kernel writes
        # every element of out, so these are never observed.
        self.outbuf = jax.device_put(
            np.zeros((B * out_shapes[0][0],) + out_shapes[0][1:], BF16),
            self.sharding)

    def run(self, fin_global: np.ndarray) -> list[np.ndarray]:
        jax = self.jax
        fin_dev = jax.device_put(fin_global, self.sharding)
        (out_g,) = self.sharded(fin_dev, self.outbuf)
        shards = sorted(out_g.addressable_shards,
                        key=lambda s: s.index[0].start or 0)
        for sh in shards:
            sh.data.copy_to_host_async()
        return [np.asarray(sh.data) for sh in shards]


def _get_rt() -> "_Runtime":
    global _rt_cache
    if _rt_cache is None:
        _rt_cache = _Runtime()
    return _rt_cache


def kernel(features_1: np.ndarray, features_2: np.ndarray) -> np.ndarray:
    f1 = np.asarray(features_1, dtype=np.float32)
    f2 = np.asarray(features_2, dtype=np.float32)
    assert f1.shape == (B, C, H, W) and f2.shape == (B, C, H, W)
    rt = _get_rt()
    fin = _pack_inputs(f1, f2)
    shards = rt.run(fin)
    return _unpack_outputs(shards)


# revision 13
# speedup vs baseline: 12650.3853x; 1.3751x over previous
"""Correlation layer (FlowNet-style cost volume) Trainium2 Bass kernel.

out[b, o, h, w] = (1/C) * sum_c f1[b,c,h,w] * f2pad[b,c,h+dy,w+dx],
o = iy*21 + ix, (dy, dx) = (2*iy, 2*ix), zero padding 20 in H and W.
B=8, C=256, H=64, W=96, 441 offsets.  Data-parallel: one batch per core.

Per core (all bf16 on the wire and in SBUF; PSUM accumulation fp32):
  - Inputs packed host-side as fin [128, 4*6144] = [f1c0|f1c1|f2c0|f2c1],
    W columns permuted even-first so matmuls split by W parity (M=48)
    skip cross-parity products (dx is always even).
  - Device pads f2 in H (zero margin rows memset once; middle rows DMAed
    straight from DRAM), so the padded rows are never uploaded.
  - PE: psum[p, (t, c')] = sum_c f1s[c, h, 2p+q] * f2p[c, h+2t, 2c'+q],
    10 dy rows batched per matmul (PSUM bank holds 480 fp32), 3 groups
    per (h, parity), accumulated over 2 C-chunks with ch-grouped issue
    order so the stationary f1 tile is loaded twice per (h, parity).
  - PSUM -> staging copies (fp32->bf16 cast) write c'-major:
    S[p, (c'+10)*21 + t], split across ScalarE (q=0) and VectorE (q=1).
  - Band extraction: the 441 needed correlations of partition p are the
    CONTIGUOUS staging run S[p, 21p : 21p+441] (element r = j*21 + t ==
    (dy=2t, dx=2j) at w=2p+q); staging has 4 h-slots, so ONE diagonal-AP
    DMA per (4 h, parity) ships straight to DRAM out (48*4 descriptors
    of 882B). Off-edge positions read staging margins memset to 0 once
    -> output exact 0, no host-side masking.
  - Output DRAM [h, parity, p, j*21+t] bf16; host reassembles/upcasts.

Execution avoids run_bass_via_pjrt's per-call zero-output upload: the
output-buffer operands live on device once and are NOT donated (the
kernel writes every output element, so their values never matter).
"""
import sys

for _p in ("/opt/trn_rl_repo", "/root/.axon_site/_ro/trn_rl_repo"):
    if _p not in sys.path:
        sys.path.insert(0, _p)

import numpy as np
import ml_dtypes

import concourse.bass as bass
import concourse.mybir as mybir
from concourse.ap import AP

BF16 = ml_dtypes.bfloat16

B, C, H, W = 8, 256, 64, 96
NOFF = 21
NCHUNK = 2
HP = H + 40                  # 104 padded f2 rows
F1SZ = H * W                 # 6144 per chunk
F2SZ = HP * W                # 9984 per chunk (padded, device side)
FINROW = 4 * F1SZ            # 24576: [f1c0|f1c1|f2c0|f2c1]
SROW = NOFF * 68             # 1428 staging cols per h-slot
HSLOT = 4                    # h-slots per staging tensor
NSLOT = 8                    # psum slots
GROUPS = [(0, 10), (10, 10), (20, 1)]  # (t0, ndy)
NG = len(GROUPS)

_rt_cache = None


def _build():
    nc = bass.Bass()
    fin = nc.declare_dram_parameter("fin", [128, FINROW], mybir.dt.bfloat16,
                                    isOutput=False)
    out = nc.declare_dram_parameter("out", [H, 2, 48, NOFF * NOFF],
                                    mybir.dt.bfloat16, isOutput=True)

    import contextlib
    ctx = contextlib.ExitStack()
    f1s = ctx.enter_context(
        nc.sbuf_tensor("f1s", [128, NCHUNK * F1SZ], mybir.dt.bfloat16))
    f2p = ctx.enter_context(
        nc.sbuf_tensor("f2p", [128, NCHUNK * F2SZ], mybir.dt.bfloat16))
    S = [ctx.enter_context(nc.sbuf_tensor(f"S{q}", [48, HSLOT * SROW],
                                          mybir.dt.bfloat16))
         for q in range(2)]
    slots = [ctx.enter_context(nc.psum_tensor(f"slot{s}", [48, 480],
                                              mybir.dt.float32))
             for s in range(NSLOT)]

    load_sem = ctx.enter_context(nc.semaphore("load_sem"))
    init_sem = ctx.enter_context(nc.semaphore("init_sem"))
    pe_sem = ctx.enter_context(nc.semaphore("pe_sem"))
    cp_sem = [ctx.enter_context(nc.semaphore(f"cp{q}")) for q in range(2)]
    band_sem = [ctx.enter_context(nc.semaphore(f"band{q}")) for q in range(2)]

    def lhsT_ap(ch, h, q):
        return AP(tensor=f1s, offset=ch * F1SZ + h * W + q * 48,
                  ap=[[NCHUNK * F1SZ, 128], [1, 48]])

    def rhs_ap(ch, h, q, t0, gn):
        off = ch * F2SZ + (h + 2 * t0) * W + q * 48
        return AP(tensor=f2p, offset=off,
                  ap=[[NCHUNK * F2SZ, 128], [2 * W, gn], [1, 48]])

    def slot_out_ap(s, gn):
        return AP(tensor=slots[s], offset=0, ap=[[480, 48], [1, gn * 48]])

    def slot_rd_ap(s, gn):
        # iterate (partition, c', t): innermost contiguous on the dst side
        return AP(tensor=slots[s], offset=0,
                  ap=[[480, 48], [1, 48], [48, gn]])

    def stage_wr_ap(q, hb, t0, gn):
        # S[p, (c'+10)*21 + t] for t in [t0, t0+gn), c' in [0,48)
        return AP(tensor=S[q], offset=hb * SROW + 10 * NOFF + t0,
                  ap=[[HSLOT * SROW, 48], [NOFF, 48], [1, gn]])

    # global matmul-group schedule: (h, q, gi)
    sched = [(h, q, gi) for h in range(H) for q in range(2)
             for gi in range(NG)]

    def copy_sem_target(g):
        """(sem_q, count): completion of the staging copy for group g."""
        h, q, gi = g // (2 * NG), (g // NG) % 2, g % NG
        return q, h * NG + gi + 1

    with nc.Block() as block:
        @block.tensor
        def _(tensor):
            tensor.wait_ge(load_sem, 48)   # f1 + 2 f2 chunk DMAs
            tensor.wait_ge(init_sem, 1)    # f2 H-margin + staging memsets
            for hq in range(H * 2):
                h, q = hq // 2, hq % 2
                g0 = hq * NG
                for ch in range(NCHUNK):
                    for gi, (t0, gn) in enumerate(GROUPS):
                        g = g0 + gi
                        s = g % NSLOT
                        if ch == 0 and g >= NSLOT:
                            qq, cnt = copy_sem_target(g - NSLOT)
                            tensor.wait_ge(cp_sem[qq], cnt)
                        mm = tensor.matmul(
                            slot_out_ap(s, gn),
                            lhsT_ap(ch, h, q),
                            rhs_ap(ch, h, q, t0, gn),
                            start=(ch == 0),
                            stop=(ch == NCHUNK - 1),
                        )
                        if ch == NCHUNK - 1:
                            mm.then_inc(pe_sem, 1)

        def copy_engine_body(eng, q):
            do_copy = getattr(eng, "copy", None) or eng.tensor_copy
            for h in range(H):
                for gi, (t0, gn) in enumerate(GROUPS):
                    g = (h * 2 + q) * NG + gi
                    s = g % NSLOT
                    if gi == 0 and h % HSLOT == 0 and h >= HSLOT:
                        eng.wait_ge(band_sem[q], 16 * (h // HSLOT))
                    eng.wait_ge(pe_sem, g + 1)
                    do_copy(stage_wr_ap(q, h % HSLOT, t0, gn),
                            slot_rd_ap(s, gn)).then_inc(cp_sem[q], 1)

        @block.scalar
        def _(scalar):
            copy_engine_body(scalar, 0)

        @block.vector
        def _(vector):
            # zero f2 H-padding margins (20 rows top/bottom per chunk) and
            # the staging (margins + everything) once: band extraction then
            # reads exact 0 at off-edge positions.
            for ch in range(NCHUNK):
                vector.memset(AP(tensor=f2p, offset=ch * F2SZ,
                                 ap=[[NCHUNK * F2SZ, 128], [1, 20 * W]]), 0.0)
                vector.memset(AP(tensor=f2p, offset=ch * F2SZ + (20 + H) * W,
                                 ap=[[NCHUNK * F2SZ, 128], [1, 20 * W]]), 0.0)
            memsets = [vector.memset(S[q][:, :], 0.0) for q in range(2)]
            memsets[-1].then_inc(init_sem, 1)
            copy_engine_body(vector, 1)

        def q_band_body(eng, q):
            with nc.allow_non_contiguous_dma(reason="band diag extraction"):
                for k in range(H // HSLOT):
                    if k == 0:
                        eng.wait_ge(init_sem, 1)  # staging margins zeroed
                    eng.wait_ge(cp_sem[q], NG * HSLOT * (k + 1))
                    src = AP(tensor=S[q], offset=0,
                             ap=[[HSLOT * SROW + NOFF, 48], [SROW, HSLOT],
                                 [1, NOFF * NOFF]])
                    dst = AP(tensor=out, offset=(HSLOT * k * 2 + q) * 48 * 441,
                             ap=[[441, 48], [2 * 48 * 441, HSLOT], [1, 441]])
                    eng.dma_start(out=dst, in_=src).then_inc(band_sem[q], 16)
                eng.wait_ge(band_sem[q], 16 * (H // HSLOT))

        @block.sync
        def _(sync):
            sync.dma_start(out=f1s[:, :],
                           in_=fin[:, 0:NCHUNK * F1SZ]).then_inc(load_sem, 16)
            for ch in range(NCHUNK):
                dst = AP(tensor=f2p, offset=ch * F2SZ + 20 * W,
                         ap=[[NCHUNK * F2SZ, 128], [1, F1SZ]])
                src = fin[:, (2 + ch) * F1SZ:(3 + ch) * F1SZ]
                sync.dma_start(out=dst, in_=src).then_inc(load_sem, 16)
            q_band_body(sync, 0)

        @block.gpsimd
        def _(gpsimd):
            q_band_body(gpsimd, 1)

    return nc


_host_fns = None


def _get_host_fns():
    """Jitted XLA-CPU pack/unpack (numpy's strided bf16 casts are ~10x
    slower than XLA's fused multithreaded gather+cast)."""
    global _host_fns
    if _host_fns is None:
        import jax
        import jax.numpy as jnp
        cpu = jax.devices("cpu")[0]

        def pack_fn(f1, f2):
            fs = f1.reshape(B, 2, 128, H, W) * np.float32(1.0 / C)
            f2r = f2.reshape(B, 2, 128, H, W)

            def parts(x):  # [B,ch,128,H,W] -> [B,ch,128,H,2par,48]
                return jnp.stack([x[..., 0::2], x[..., 1::2]], axis=4)

            finv = jnp.concatenate([parts(fs), parts(f2r)], axis=1)
            finv = finv.transpose(0, 2, 1, 3, 4, 5)   # [B,128,4,H,2,48]
            return finv.reshape(B * 128, FINROW).astype(jnp.bfloat16)

        def unpack_fn(o):  # [B,64,2,48,441] bf16 -> [B,441,64,96] fp32
            ov = o.reshape(B, H, 2, 48, NOFF, NOFF).astype(jnp.float32)
            return ov.transpose(0, 5, 4, 1, 3, 2).reshape(
                B, NOFF * NOFF, H, W)  # o = t*21+j, w = 2p+q

        _host_fns = (jax.jit(pack_fn, device=cpu),
                     jax.jit(unpack_fn, device=cpu))
    return _host_fns


def _pack_inputs(f1: np.ndarray, f2: np.ndarray):
    """Full fp32 inputs [8,256,64,96] -> fin global [8*128, FINROW] bf16."""
    pack_fn, _ = _get_host_fns()
    return pack_fn(f1, f2)


def _unpack_outputs(shards: list[np.ndarray]) -> np.ndarray:
    """Per-core [64, 2, 48, 441] bf16 (441 = j*21+t) -> [8,441,64,96] fp32."""
    _, unpack_fn = _get_host_fns()
    return np.asarray(unpack_fn(np.stack(shards)))


class _Runtime:
    def __init__(self):
        import jax
        from jax.experimental.shard_map import shard_map
        from jax.sharding import Mesh, PartitionSpec, NamedSharding
        from concourse.bass2jax import install_neuronx_cc_hook, _bass_exec_p
        from concourse.bass2jax import partition_id_tensor

        self.jax = jax
        install_neuronx_cc_hook()
        nc = self.nc = _build()

        in_names, out_names, out_avals, out_shapes = [], [], [], []
        for alloc in nc.m.functions[0].allocations:
            if not isinstance(alloc, mybir.MemoryLocationSet):
                continue
            name = alloc.memorylocations[0].name
            if alloc.kind == "ExternalInput":
                if (nc.partition_id_tensor is None
                        or name != nc.partition_id_tensor.name):
                    in_names.append(name)
            elif alloc.kind == "ExternalOutput":
                shape = tuple(alloc.tensor_shape)
                out_names.append(name)
                out_shapes.append(shape)
                out_avals.append(
                    jax.core.ShapedArray(shape, mybir.dt.np(alloc.dtype)))
        n_params = len(in_names)
        in_names = in_names + out_names
        if nc.partition_id_tensor is not None:
            in_names.append(nc.partition_id_tensor.name)

        def _body(*args):
            operands = list(args)
            if nc.partition_id_tensor is not None:
                operands.append(partition_id_tensor())
            outs = _bass_exec_p.bind(
                *operands,
                out_avals=tuple(out_avals),
                in_names=tuple(in_names),
                out_names=tuple(out_names),
                lowering_input_output_aliases=(),
                sim_require_finite=True,
                sim_require_nnan=True,
                nc=nc,
            )
            return tuple(outs)

        devices = jax.devices()[:B]
        assert len(devices) == B
        mesh = Mesh(np.asarray(devices), ("core",))
        self.sharding = NamedSharding(mesh, PartitionSpec("core"))
        nin = n_params + len(out_names)

        self.sharded = jax.jit(
            shard_map(_body, mesh=mesh,
                      in_specs=(PartitionSpec("core"),) * nin,
                      out_specs=(PartitionSpec("core"),) * len(out_names),
                      check_rep=False),
            keep_unused=True,
        )
        # Persistent (non-donated) output-buffer operands: the kernel writes
        # every element of out, so these are never observed.
        self.outbuf = jax.device_put(
            np.zeros((B * out_shapes[0][0],) + out_shapes[0][1:], BF16),
            self.sharding)

    def run(self, fin_global: np.ndarray) -> list[np.ndarray]:
        jax = self.jax
        fin_dev = jax.device_put(fin_global, self.sharding)
        (out_g,) = self.sharded(fin_dev, self.outbuf)
        shards = sorted(out_g.addressable_shards,
                        key=lambda s: s.index[0].start or 0)
        for sh in shards:
            sh.data.copy_to_host_async()
        return [np.asarray(sh.data) for sh in shards]


def _get_rt() -> "_Runtime":
    global _rt_cache
    if _rt_cache is None:
        _rt_cache = _Runtime()
    return _rt_cache


def kernel(features_1: np.ndarray, features_2: np.ndarray) -> np.ndarray:
    f1 = np.asarray(features_1, dtype=np.float32)
    f2 = np.asarray(features_2, dtype=np.float32)
    assert f1.shape == (B, C, H, W) and f2.shape == (B, C, H, W)
    rt = _get_rt()
    fin = _pack_inputs(f1, f2)
    shards = rt.run(fin)
    return _unpack_outputs(shards)
